# revision 18
# baseline (speedup 1.0000x reference)
"""Trainium2 Bass kernel for nn_EndToEndBertBrain.

Pipeline (per story): Lanczos downsample -> trim -> z-score; concat stories;
4-delay feature expansion; linear regression to 50k voxels.

Distribution: stages up to the feature matrix are replicated on all 8 cores
(cheap); the regression weight W [4096, 50000] and the prediction columns are
sharded over voxels, 6250 per core (tensor parallel on the output dim).

All matmuls run as float32r (FP22 single-pass, full PE rate at N>=256).
The feature matrix is kept transposed ([feature, time]) end to end so that
row-normalization, z-scoring and the big matmul's lhsT tiles all fall on
native engine axes; the delay expansion is pure AP slicing over a 3-column
zero pad. The natural-layout `delayed` output is produced by PE transposes
plus shifted DMA writes.
"""
import numpy as np
from contextlib import ExitStack

import concourse.bass as bass
import concourse.bacc as bacc
import concourse.mybir as mybir
import concourse.tile as tile
from concourse import bass_utils
from concourse.masks import make_identity

dt = mybir.dt
F32, F32R = dt.float32, dt.float32r
AF = mybir.ActivationFunctionType
ALU = mybir.AluOpType

# problem shapes (fixed by the harness)
S, NSRC, D = 4, 2000, 1024
NTR, NVOX = 300, 50000
TRIM = 10
T = NTR - 2 * TRIM           # 280
N = S * T                    # 1120
D4 = 4 * D                   # 4096
NCORES = 8
VC = NVOX // NCORES          # 6250

KP = 125                     # stage-A contraction tile (2000 = 16*125)
NKT = NSRC // KP             # 16
BAND = 28                    # sinc support band width (actual support ~22)
BSTEP = 19                   # band start slope: band of k-tile k = tr indices
BOFF = -6                    # [19k-6, 19k+21]  (support is [18.76k-1.6, 18.76k+20.3])
GW = 313                     # group width in wT_big: 6 junk + 300 tr + 7 junk
NT = 384                     # big-matmul N tile
M_TILES = [(m * 128, min(128, N - m * 128)) for m in range((N + 127) // 128)]

PI = float(np.float32(np.pi))
FLPI = float(np.float32(np.pi))


def band_cols(k):
    """tr-index range [i0, i1) of k-tile k's band, clipped to valid tr."""
    i0 = BSTEP * k + BOFF
    return max(i0, 0), min(i0 + BAND, NTR)


def band_cols_aligned(k):
    """band_cols with i0 rounded down to 16 (psum matmul offset alignment)
    and the width rounded up to even (f32r matmul width constraint); the
    extra columns read zero-filled wT and contribute nothing."""
    i0, i1 = band_cols(k)
    a0 = (i0 // 16) * 16
    if (i1 - a0) % 2:
        i1 += 1
    return a0, min(i1, NTR)


def build():
    nc = bacc.Bacc("TRN2", target_bir_lowering=False)

    emb = nc.dram_tensor("embeddings", [S, NSRC, D], F32, kind="ExternalInput")
    data_t = nc.dram_tensor("data_times", [S, NSRC], F32, kind="ExternalInput")
    tr_t = nc.dram_tensor("tr_times", [S, NTR], F32, kind="ExternalInput")
    Wsh = nc.dram_tensor("w_shard", [D4, VC], F32, kind="ExternalInput")
    bsh = nc.dram_tensor("b_shard", [1, VC], F32, kind="ExternalInput")

    preds = nc.dram_tensor("preds", [N, VC], F32, kind="ExternalOutput")
    delayed = nc.dram_tensor("delayed", [N, D4], F32, kind="ExternalOutput")

    with tile.TileContext(nc) as tc, ExitStack() as ctx:
        # --- pools (W first so its addresses never alias released pools) ---
        wp = ctx.enter_context(tc.tile_pool(name="wp", bufs=48))
        fpp = ctx.enter_context(tc.tile_pool(name="fpp", bufs=1))
        outp = ctx.enter_context(tc.tile_pool(name="outp", bufs=3))
        bp = ctx.enter_context(tc.tile_pool(name="bp", bufs=2))
        fnat = ctx.enter_context(tc.tile_pool(name="fnat", bufs=2))
        cst = ctx.enter_context(tc.tile_pool(name="cst", bufs=1))

        wtp = ctx.enter_context(tc.tile_pool(name="wtp", bufs=2))
        embp = ctx.enter_context(tc.tile_pool(name="embp", bufs=6))
        stry = ctx.enter_context(tc.tile_pool(name="stry", bufs=2))
        scr = ctx.enter_context(tc.tile_pool(name="scr", bufs=1))
        tmpp = ctx.enter_context(tc.tile_pool(name="tmpp", bufs=3))
        zc = ctx.enter_context(tc.tile_pool(name="zc", bufs=4))
        # single psum pool: sp(1) + dp(2) + tp(2) + out(3) = 8 banks, no
        # pool-release barrier between stage A and the big matmul
        psum = ctx.enter_context(tc.tile_pool(name="psum", bufs=1, space="PSUM"))

        # --- constants ---
        ident = cst.tile([128, 128], F32)
        make_identity(nc, ident)
        onesf = cst.tile([KP, 128], F32)
        nc.vector.memset(onesf, 1.0)
        ones_r = cst.tile([KP, 128], F32R)
        nc.vector.tensor_scalar(out=ones_r, in0=onesf, scalar1=1.0, scalar2=None,
                                op0=ALU.mult)
        three = cst.tile([128, 1], F32)
        nc.vector.memset(three, 3.0)
        zrow = cst.tile([3, 1024], F32)
        nc.vector.memset(zrow, 0.0)

        # per-story scalars, broadcast to all partitions via DRAM-source DMAs:
        # upi[s] = pi * (data_end - data_start) / (tr_end - tr_start)
        dstart = cst.tile([128, S], F32)
        nc.gpsimd.dma_start(out=dstart, in_=data_t[:, 0:1]
                            .rearrange("s one -> one s").to_broadcast([128, S]))
        dend = cst.tile([128, S], F32)
        nc.gpsimd.dma_start(out=dend, in_=data_t[:, NSRC - 1:NSRC]
                            .rearrange("s one -> one s").to_broadcast([128, S]))
        tstart = cst.tile([128, S], F32)
        nc.gpsimd.dma_start(out=tstart, in_=tr_t[:, 0:1]
                            .rearrange("s one -> one s").to_broadcast([128, S]))
        tend = cst.tile([128, S], F32)
        nc.gpsimd.dma_start(out=tend, in_=tr_t[:, NTR - 1:NTR]
                            .rearrange("s one -> one s").to_broadcast([128, S]))
        num = cst.tile([128, S], F32)
        nc.vector.tensor_tensor(out=num, in0=dend, in1=dstart, op=ALU.subtract)
        den = cst.tile([128, S], F32)
        nc.vector.tensor_tensor(out=den, in0=tend, in1=tstart, op=ALU.subtract)
        rden = cst.tile([128, S], F32)
        nc.vector.reciprocal(out=rden, in_=den)
        scl = cst.tile([128, S], F32)
        nc.vector.tensor_tensor(out=scl, in0=num, in1=rden, op=ALU.mult)
        # upi3[s] = pi/3 * scale_s   (v = pi*x/3 formulation)
        upi3 = cst.tile([128, S], F32)
        nc.vector.tensor_scalar(out=upi3, in0=scl, scalar1=float(np.float32(PI / 3)),
                                scalar2=None, op0=ALU.mult)

        # --- feature pad: 8 f-blocks of [128, 3 + N] (f32r), 3 zero lead cols
        featpad = []
        for f in range(8):
            fp = fpp.tile([128, 3 + N], F32R, tag=f"fp{f}", name=f"featpad{f}")
            nc.vector.tensor_scalar(out=fp[:, 0:3], in0=ident[:, 0:3],
                                    scalar1=0.0, scalar2=None, op0=ALU.mult)
            featpad.append(fp)

        # =================== stage A: per-story features ===================
        # wT is one [KP, 16*GW] tile per story: group k holds the k-tile's
        # [KP, 300] weight block at cols [GW*k+6, GW*k+306); the sinc band of
        # group k sits at the affine range [332k, 332k+28) so all 16 k-tiles
        # are produced by ONE strided op chain.
        for s in range(S):
            trbp = stry.tile([128, 320], F32, tag="trbp")
            nc.vector.memset(trbp, 0.0)
            nc.gpsimd.dma_start(out=trbp[:, 6:6 + NTR], in_=tr_t[s:s + 1, :]
                                .to_broadcast([128, NTR]))
            trv = stry.tile([128, 320], F32, tag="trv")
            nc.vector.tensor_scalar(out=trv, in0=trbp, scalar1=upi3[:, s:s + 1],
                                    scalar2=None, op0=ALU.mult)
            dsc = stry.tile([KP, NKT], F32, tag="dsc")
            nc.sync.dma_start(out=dsc, in_=data_t[s, :]
                              .rearrange("(t p) -> p t", p=KP))
            dscv = stry.tile([KP, NKT], F32, tag="dscv")
            nc.vector.tensor_scalar(out=dscv, in0=dsc, scalar1=upi3[:KP, s:s + 1],
                                    scalar2=None, op0=ALU.mult)

            wT = wtp.tile([KP, NKT * GW], F32R, tag="wt", name=f"wt{s}")
            # zero-fill the whole weight tile (banded matmul windows are
            # 16-aligned and can reach a few junk columns left of the band)
            wT3z = wT.rearrange("p (g c) -> p g c", g=NKT)
            trz = bass.AP(tensor=trbp.tensor, offset=trbp.offset,
                          ap=[[trbp.ap[0][0], KP], [0, NKT], [1, GW]])
            nc.vector.tensor_scalar(out=wT3z, in0=trz, scalar1=0.0,
                                    scalar2=None, op0=ALU.mult)

            # banded op chain over all 16 groups at once:
            # v = clamp(|pi/3*(tr_i - data_j)*scale|), band (k, j): tr col 19k+j
            trg = bass.AP(tensor=trv.tensor, offset=trv.offset,
                          ap=[[trv.ap[0][0], KP], [BSTEP, NKT], [1, BAND]])
            dscb = dscv.rearrange("p (g one) -> p g one", one=1) \
                .to_broadcast([KP, NKT, BAND])
            vr = scr.tile([KP, NKT * BAND], F32, tag="sA")
            vr3 = vr.rearrange("p (g c) -> p g c", g=NKT)
            nc.vector.tensor_tensor(out=vr3, in0=trg, in1=dscb, op=ALU.subtract)
            av = scr.tile([KP, NKT * BAND], F32, tag="sB")
            nc.scalar.activation(out=av, in_=vr, func=AF.Abs)
            v = scr.tile([KP, NKT * BAND], F32, tag="sA")
            nc.gpsimd.tensor_scalar(out=v, in0=av, scalar1=3e-4, scalar2=FLPI,
                                    op0=ALU.max, op1=ALU.min)
            sn = scr.tile([KP, NKT * BAND], F32, tag="sB")
            nc.scalar.activation(out=sn, in_=v, func=AF.Sin)
            d2 = scr.tile([KP, NKT * BAND], F32, tag="sC")
            nc.scalar.activation(out=d2, in_=v, func=AF.Square)
            s2 = scr.tile([KP, NKT * BAND], F32, tag="sA")
            nc.vector.tensor_tensor(out=s2, in0=sn, in1=sn, op=ALU.mult)
            # q = 3 - 4*sin^2  (triple angle: sin(3v) = sin(v)*q)
            q = scr.tile([KP, NKT * BAND], F32, tag="sB")
            nc.scalar.activation(out=q, in_=s2, func=AF.Identity,
                                 bias=three[:KP, :], scale=-4.0)
            r = scr.tile([KP, NKT * BAND], F32, tag="sD")
            rscratch = scr.tile([KP, NKT * BAND], F32, tag="sE")
            nc.vector.reciprocal_approx_accurate(out=r, in_=d2, scratch=rscratch)
            t = scr.tile([KP, NKT * BAND], F32, tag="sC")
            nc.vector.tensor_tensor(out=t, in0=s2, in1=r, op=ALU.mult)
            # w = q * t / 3 into the affine band columns of wT (col 332k + j)
            wband = bass.AP(tensor=wT.tensor, offset=wT.offset,
                            ap=[[wT.ap[0][0], KP], [GW + BSTEP, NKT], [1, BAND]])
            nc.vector.scalar_tensor_tensor(
                out=wband, in0=q.rearrange("p (g c) -> p g c", g=NKT),
                scalar=float(np.float32(1 / 3)),
                in1=t.rearrange("p (g c) -> p g c", g=NKT),
                op0=ALU.mult, op1=ALU.mult)

            # -- row sums (broadcast to 128 partitions via ones lhsT) --
            sp = psum.tile([128, NTR], F32, tag="sp", bufs=1, name="spsum")
            nc.tensor.matmul(sp, ones_r, wT[:, 6:6 + NTR], start=True, stop=False)
            for k in range(1, NKT):
                i0, i1 = band_cols_aligned(k)
                nc.tensor.matmul(sp[:, i0:i1], ones_r,
                                 wT[:, GW * k + 6 + i0:GW * k + 6 + i1],
                                 start=False, stop=(k == NKT - 1))
            invb = stry.tile([128, NTR], F32, tag="invb")
            nc.vector.reciprocal(out=invb, in_=sp)

            # -- downsample matmuls, f-outer so only 2 psum banks cycle;
            # -- evac + z-score of each f overlaps the next f's accumulation
            for f in range(8):
                dp = psum.tile([128, NTR], F32, tag="dp", bufs=2,
                               name=f"dp{s}_{f}")
                for k in range(NKT):
                    et = embp.tile([KP, 128], F32R, tag="emb")
                    nc.sync.dma_start(
                        out=et, in_=emb[s, k * KP:(k + 1) * KP,
                                        f * 128:(f + 1) * 128].bitcast(F32R))
                    if k == 0:
                        nc.tensor.matmul(dp, et, wT[:, 6:6 + NTR],
                                         start=True, stop=False)
                    else:
                        i0, i1 = band_cols_aligned(k)
                        nc.tensor.matmul(dp[:, i0:i1], et,
                                         wT[:, GW * k + 6 + i0:GW * k + 6 + i1],
                                         start=False, stop=(k == NKT - 1))

                tmp = tmpp.tile([128, NTR], F32, tag="tmp")
                nc.vector.tensor_tensor(out=tmp, in0=dp, in1=invb,
                                        op=ALU.mult)
                st6 = zc.tile([128, 6], F32, tag="st6")
                nc.vector.bn_stats(out=st6, in_=tmp[:, TRIM:NTR - TRIM])
                mv = zc.tile([128, 2], F32, tag="mv")
                nc.vector.bn_aggr(out=mv, in_=st6)
                sd = zc.tile([128, 1], F32, tag="sd")
                nc.scalar.activation(out=sd, in_=mv[:, 1:2], func=AF.Sqrt,
                                     scale=float(np.float32(T / (T - 1))))
                sde = zc.tile([128, 1], F32, tag="sde")
                nc.gpsimd.tensor_scalar(out=sde, in0=sd, scalar1=1e-6,
                                        scalar2=None, op0=ALU.add)
                rstd = zc.tile([128, 1], F32, tag="rstd")
                nc.vector.reciprocal(out=rstd, in_=sde)
                nmr = zc.tile([128, 1], F32, tag="nmr")
                nc.vector.scalar_tensor_tensor(out=nmr, in0=mv[:, 0:1],
                                               scalar=-1.0, in1=rstd,
                                               op0=ALU.mult, op1=ALU.mult)
                nc.scalar.activation(out=featpad[f][:, 3 + s * T:3 + (s + 1) * T],
                                     in_=tmp[:, TRIM:NTR - TRIM], func=AF.Identity,
                                     bias=nmr, scale=rstd)

        # =================== delayed output ===================
        for (m0, msz) in M_TILES:
            fn = fnat.tile([128, D], F32, tag="fn")
            for f in range(8):
                tp = psum.tile([128, 128], F32, tag="tp", bufs=2)
                nc.tensor.transpose(tp[:msz, :],
                                    featpad[f][:, 3 + m0:3 + m0 + msz].bitcast(F32),
                                    ident)
                if f % 2 == 0:
                    nc.scalar.activation(out=fn[:msz, f * 128:(f + 1) * 128],
                                         in_=tp[:msz, :], func=AF.Copy)
                else:
                    nc.vector.tensor_copy(fn[:msz, f * 128:(f + 1) * 128],
                                          tp[:msz, :])
            for k in range(4):
                rows = min(msz, N - k - m0)
                if rows > 0:
                    nc.sync.dma_start(
                        out=delayed[m0 + k:m0 + k + rows, k * D:(k + 1) * D],
                        in_=fn[0:rows, :])
        for k in range(1, 4):
            nc.sync.dma_start(out=delayed[0:k, k * D:(k + 1) * D],
                              in_=zrow[0:k, :])

        # =================== big matmul: preds = delayed @ W + b ===========
        n_blocks = []
        n0 = 0
        while n0 < VC:
            nsz = min(NT, VC - n0)
            n_blocks.append((n0, nsz))
            n0 += nsz

        for (n0, nsz) in n_blocks:
            wts2 = []
            for kk in range(32):
                dly, f = divmod(kk, 8)
                wrow = dly * D + f * 128
                wtile = wp.tile([128, NT], F32R, tag="w", name=f"w{n0}_{kk}")
                nc.sync.dma_start(out=wtile[:, 0:nsz],
                                  in_=Wsh[wrow:wrow + 128, n0:n0 + nsz]
                                  .bitcast(F32R))
                wts2.append(wtile)
            bt = bp.tile([128, NT], F32, tag="b")
            nc.gpsimd.dma_start(out=bt[:, 0:nsz],
                                in_=bsh[0:1, n0:n0 + nsz].to_broadcast([128, nsz]))
            for mi, (m0, msz) in enumerate(M_TILES):
                ps = psum.tile([128, NT], F32, tag="out", bufs=3)
                for kk in range(32):
                    dly, f = divmod(kk, 8)
                    lhsT = featpad[f][:, 3 - dly + m0:3 - dly + m0 + msz]
                    nc.tensor.matmul(ps[:msz, 0:nsz], lhsT, wts2[kk][:, 0:nsz],
                                     start=(kk == 0), stop=(kk == 31))
                ot = outp.tile([128, NT], F32, tag="o")
                nc.vector.tensor_tensor(out=ot[:msz, 0:nsz], in0=ps[:msz, 0:nsz],
                                        in1=bt[:msz, 0:nsz], op=ALU.add)
                nc.sync.dma_start(out=preds[m0:m0 + msz, n0:n0 + nsz],
                                  in_=ot[:msz, 0:nsz])

    nc.finalize()
    return nc


_NC = None


def _get_nc():
    global _NC
    if _NC is None:
        _NC = build()
    return _NC


def kernel(embeddings, data_times, tr_times, W, b):
    embeddings = np.ascontiguousarray(embeddings, dtype=np.float32)
    data_times = np.ascontiguousarray(data_times, dtype=np.float32)
    tr_times = np.ascontiguousarray(tr_times, dtype=np.float32)
    W = np.asarray(W, dtype=np.float32)
    b = np.asarray(b, dtype=np.float32)

    in_maps = []
    for c in range(NCORES):
        in_maps.append({
            "embeddings": embeddings,
            "data_times": data_times,
            "tr_times": tr_times,
            "w_shard": np.ascontiguousarray(W[:, c * VC:(c + 1) * VC]),
            "b_shard": np.ascontiguousarray(b[c * VC:(c + 1) * VC]).reshape(1, VC),
        })

    res = bass_utils.run_bass_kernel_spmd(_get_nc(), in_maps,
                                          core_ids=list(range(NCORES)))
    preds = np.concatenate(
        [np.asarray(res.results[c]["preds"], np.float32) for c in range(NCORES)],
        axis=1)
    delayed = np.asarray(res.results[0]["delayed"], np.float32)
    return preds, delayed
